# revision 38
# baseline (speedup 1.0000x reference)
"""Bloom transformer block on 8 Trainium2 NeuronCores.

Sharding: core c handles batch c//4 and 512 of its 2048 tokens (two
causally-balanced 256-token q-tiles {r, r+4}, r = c%4).  Each core computes
LN1 + Q/K/V projections for ONLY its own 512 tokens; K^T and V slices are
then shared within each 4-core batch group via chunked AllGather collectives
(one per 4-head group, interleaved with compute so the cc stream hides under
projection/attention).  Attention/o_proj/MLP run on own tokens only.  The
diagonal attention k-tiles (own tokens) are read from local SBUF at fixed
program positions, so the triangular causal masks stay compile-time; all
remaining rank-dependence lives in host-prepared acol/eacol data (alibi
columns + masking of invalid gathered tiles).  The device program is
identical across cores (SPMD).  The host concatenates the output slices.

Softmax: scores^T layout [k, q], no max-subtraction (scores are bounded).
Off-diagonal tiles use the factored form
    es = exp(s) * exp(acol) * exp(-slope*qq)
(a per-k-partition column factor applied on gpsimd and a row-tile factor
applied on vector, both in bf16 after a batched scalar-engine exp straight
from PSUM); masked tiles get exp(NEG)=0 through the column factor.
Diagonal tiles keep the exact pre-exp path (vector scalar_tensor_tensor
with the triangular mask row tiles) since their column term is positive
and would overflow the factored exp.

Exact math shortcuts: the k-projection bias adds a per-query constant to
every score row, so it cancels in softmax and is dropped; the v-projection
bias adds exactly bv to each attention output (probs sum to 1), so bv @ wo
is folded into the host-prepared residual.  LN affine params are folded
into the following matmul weights; 1/sqrt(HD) is folded into Wq.
"""

import math
import os

import numpy as np
import ml_dtypes

import concourse.bass as bass
import concourse.tile as tile
from concourse import mybir
from concourse.bass_utils import run_bass_kernel_spmd

B, S, D, H = 2, 2048, 2048, 16
HD = D // H          # 128
FF = 4 * D           # 8192
EPS = 1e-5
NCORES = 8
GS = 4               # cores per batch (group size)
QT = S // GS         # own tokens per core = 512
NEG = -1.0e9
QW = 256             # q-tile width (tokens) in attention
NSLOT = QT // QW     # 2 slots per core
NKT = S // 128       # 16 k-tiles
DT16 = D // 128      # 16 feature tiles of 128
FT64 = FF // 128     # 64 FF tiles
QSCALE = 1.0 / math.sqrt(HD)
GROUPS = [[0, 1, 2, 3], [4, 5, 6, 7]]
# per-slot read extents (in 128-wide k-tiles): 2 local diag + gathered
KEXT = [8, 16]
SK = sum(KEXT)       # 24 acol columns per head

f32 = mybir.dt.float32
bf16 = mybir.dt.bfloat16
f8 = mybir.dt.float8e3


def _alibi_slopes(num_heads):
    closest = 2 ** math.floor(math.log2(num_heads))
    base = 2.0 ** (-(2.0 ** (-(math.log2(closest) - 3))))
    powers = np.arange(1, 1 + closest, dtype=np.float64)
    slopes = base ** powers
    if closest != num_heads:
        extra_base = 2.0 ** (-(2.0 ** (-(math.log2(2 * closest) - 3))))
        num_rem = min(closest, num_heads - closest)
        extra_powers = np.arange(1, 1 + 2 * num_rem, 2, dtype=np.float64)
        slopes = np.concatenate([slopes, extra_base ** extra_powers])
    return slopes.astype(np.float32)


# ---------------------------------------------------------------------------
# wait-split post-pass: this walrus build supports a single sync-wait per
# instruction; excess waits move onto preceding NoOps on the same engine.
# ---------------------------------------------------------------------------
_ctr = [0]


def _split_waits(nc, maxw=1):
    for f in nc.m.functions:
        for bb in f.blocks:
            out = []
            changed = False
            for ins in bb.instructions:
                si = ins.sync_info
                waits = list(si.on_wait) if (si and si.on_wait) else []
                if len(waits) > maxw:
                    head, keep = waits[:-maxw], waits[-maxw:]
                    for w in head:
                        _ctr[0] += 1
                        nop = mybir.InstNoOp(name=f"I-waitsplit-{_ctr[0]}")
                        nop.engine = ins.engine
                        nop.sync_info = mybir.SyncInfo(on_wait=[w], on_update=[])
                        out.append(nop)
                    si.on_wait = keep
                    changed = True
                out.append(ins)
            if changed:
                bb.instructions = out
    return nc


# ---------------------------------------------------------------------------
# attention p-loop read map (program-level, rank-independent)
# entries: ("loc", col0) local kt_own column, or ("rr", rank, col0) gathered
# tile kt_sb[:, rank, col0:col0+128].  var: 0 plain (fast exp path),
# 1/2 diag triangle (exact pre-exp path).
# ---------------------------------------------------------------------------
def _read_map():
    plan = []                     # per slot: list of (src, var)
    s0 = [(("loc", 0), 1), (("loc", 128), 2)]
    for rr in range(3):
        for half in range(2):
            s0.append((("rr", rr, half * 128), 0))
    plan.append(s0)
    s1 = [(("loc", 256), 1), (("loc", 384), 2)]
    for rr in range(4):
        for half in range(2):
            s1.append((("rr", rr, half * 128), 0))
    for rr in range(3):
        for half in range(2):
            s1.append((("rr", rr, 256 + half * 128), 0))
    plan.append(s1)
    return plan


READ_MAP = _read_map()
# fast-path (var0) positions per slot, grouped in pairs for batched exp
FAST_PAIRS = []
for _j in range(NSLOT):
    _fp = [p for p, (_s, _v) in enumerate(READ_MAP[_j]) if _v != 0]
    _fa = [p for p, (_s, _v) in enumerate(READ_MAP[_j]) if _v == 0]
    FAST_PAIRS.append((_fp, [(_fa[i], _fa[i + 1])
                             for i in range(0, len(_fa), 2)]))


def _build_acol(r, slopes):
    """Per-core [128, H*SK] f32 column tiles: value slope_h*(k_orig - q_base)
    per in-tile k position, or NEG for masked (beyond-causal or
    locally-handled) gathered tiles."""
    out = np.empty((128, H * SK), dtype=np.float32)
    kk = np.arange(128, dtype=np.float64)
    for h in range(H):
        for j in range(NSLOT):
            qtile = r + 4 * j               # orig 256-q-tile index
            qbase = qtile * QW
            off = h * SK + sum(KEXT[:j])
            for p, (src, var) in enumerate(READ_MAP[j]):
                if src[0] == "loc":
                    # own diag 128-k-tile: src[1] is the kt_own column
                    # offset; in-slot k offset is src[1] % 256
                    out[:, off + p] = slopes[h] * (src[1] % 256 + kk)
                else:
                    _, rr, col0 = src
                    # gathered tile: rank rr's token col0 (0:256 -> orig
                    # 256-tile rr; 256:512 -> orig rr+4)
                    otile = rr if col0 < 256 else rr + 4
                    k0 = otile * QW + (col0 % 256)
                    # active iff strictly before own q-tile (diagonal is
                    # handled locally; beyond-causal masked)
                    if otile < qtile:
                        out[:, off + p] = slopes[h] * (k0 + kk - qbase)
                    else:
                        out[:, off + p] = NEG
    return out


def _build_arow(slopes):
    """Shared [H, 2, 128, QW] f32 row tiles for the diagonal (exact) path:
    var 1/2 = slope_h*qq + 1e9 on the causally-masked cells of the two
    own-diagonal k-tiles (kk > qq, 128+kk > qq)."""
    out = np.empty((H, 2, 128, QW), dtype=np.float32)
    kk = np.arange(128)
    qq = np.arange(QW)
    m0 = (kk[:, None] > qq[None, :]).astype(np.float32) * (-NEG)
    m1 = ((kk[:, None] + 128) > qq[None, :]).astype(np.float32) * (-NEG)
    for h in range(H):
        plain = np.broadcast_to(slopes[h] * qq[None, :].astype(np.float32),
                                (128, QW))
        out[h, 0] = plain + m0
        out[h, 1] = plain + m1
    return out


def _build_earow(slopes):
    """Shared [H, 128, 2*QW] bf16: exp(-slope_h*qq) tiled twice along free
    (row factor for a batch-2 fast-path es tile)."""
    qq = np.arange(QW, dtype=np.float64)
    out = np.empty((H, 128, 2 * QW), dtype=np.float64)
    for h in range(H):
        e = np.exp(-float(slopes[h]) * qq)
        out[h, :, :QW] = e[None, :]
        out[h, :, QW:] = e[None, :]
    return out.astype(ml_dtypes.bfloat16)


# ---------------------------------------------------------------------------
# device program (identical for all cores)
# ---------------------------------------------------------------------------
def build_nc():
    nc = bass.Bass(target_bir_lowering=False)

    xp = nc.dram_tensor("xp", [QT, D], bf16, kind="ExternalInput")
    xres = nc.dram_tensor("xres", [QT, D], f32, kind="ExternalInput")
    wk_p = nc.dram_tensor("wk_p", [4, DT16, 128, 512], bf16,
                          kind="ExternalInput")
    wq_p = nc.dram_tensor("wq_p", [DT16, 128, DT16, 128], bf16,
                          kind="ExternalInput")
    wv_p = nc.dram_tensor("wv_p", [4, 2, 128, DT16, 256], bf16,
                          kind="ExternalInput")
    bq_pp = nc.dram_tensor("bq_pp", [128, DT16], f32, kind="ExternalInput")
    wo = nc.dram_tensor("wo", [4, 128, DT16, 512], bf16, kind="ExternalInput")
    w1 = nc.dram_tensor("w1", [FT64, 128, DT16, 128], bf16,
                        kind="ExternalInput")
    b1_pp = nc.dram_tensor("b1_pp", [128, FT64], f32, kind="ExternalInput")
    w2 = nc.dram_tensor("w2", [4, 4, 128, 16, 512], bf16,
                        kind="ExternalInput")
    b2_bc = nc.dram_tensor("b2_bc", [128, D], f32, kind="ExternalInput")
    acol = nc.dram_tensor("acol", [128, H * SK], f32, kind="ExternalInput")
    arow = nc.dram_tensor("arow", [H, 2, 128, QW], f32, kind="ExternalInput")
    earow = nc.dram_tensor("earow", [H, 128, 2 * QW], bf16,
                           kind="ExternalInput")

    out = nc.dram_tensor("out", [QT, D], f32, kind="ExternalOutput")

    kT_local = nc.dram_tensor("kT_local", [D, QT], f8, kind="Internal")
    v_chunks = nc.dram_tensor("v_chunks", [4, QT, 512], f8, kind="Internal")
    kT_full = [nc.dram_tensor(f"kT_full{g}", [4 * 512, QT], f8,
                              kind="Internal") for g in range(4)]
    v_full = [nc.dram_tensor(f"v_full{g}", [4 * QT, 512], f8,
                             kind="Internal") for g in range(4)]
    x2_dram = nc.dram_tensor("x2_dram", [QT, D], f32, kind="Internal")
    rec_dram = nc.dram_tensor("rec_dram", [NSLOT * H, QW], f32, kind="Internal")
    ident_dram = nc.inline_tensor(np.eye(128, dtype=ml_dtypes.bfloat16),
                                  name="ident128")

    with tile.TileContext(nc) as tc:
        with tc.tile_pool(name="persist", bufs=1) as pp:
            ones = pp.tile([128, 1], bf16, tag="ones")
            nc.vector.memset(ones, 1.0)
            eps_t = pp.tile([128, 1], f32, tag="eps")
            nc.vector.memset(eps_t, EPS)
            ident = pp.tile([128, 128], bf16, tag="ident")
            nc.sync.dma_start(out=ident, in_=ident_dram.ap())
            b1_sb = pp.tile([128, FT64], f32, tag="b1")
            nc.scalar.dma_start(out=b1_sb, in_=b1_pp.ap())
            # bf16 mirror of x2 for LN2 (avoids the DRAM roundtrip before
            # phase3); the f32 residual still comes from x2_dram.
            x2_sb = pp.tile([128, 4, D], bf16, tag="x2sb")
            # LN2 output (transposed) — produced inside phase2 so it
            # overlaps o_proj; consumed by MLP1 in phase3.
            h2T = pp.tile([128, DT16, QT], bf16, tag="h2T")

            with tc.tile_pool(name="attn_persist", bufs=1) as app:
                qT = [app.tile([128, QT], bf16, tag=f"qT{m}", name=f"qT{m}")
                      for m in range(DT16)]
                kt_own = [app.tile([128, QT], bf16, tag=f"ktown{m}",
                                   name=f"ktown{m}") for m in range(DT16)]
                # v_own[nch][t]: [128 tok, 512 feat]
                v_own = [[app.tile([128, 512], bf16, tag=f"vown{n}_{t}",
                                   name=f"vown{n}_{t}") for t in range(4)]
                         for n in range(4)]
                # attention tables live in the persistent pool so their DMAs
                # don't wait on phase1 SBUF frees (loaded at kernel start).
                acol_sb = app.tile([128, H * SK], f32, tag="acol")
                nc.sync.dma_start(out=acol_sb, in_=acol.ap())

                _phase1(nc, tc, xp, kT_local, v_chunks, kT_full,
                        v_full, wk_p, wq_p, wv_p, bq_pp, eps_t, ident, qT,
                        kt_own, v_own)
                _phase2(nc, tc, app, qT, kt_own, v_own, kT_full, v_full,
                        acol_sb, arow, earow, wo, xres, x2_dram,
                        x2_sb, h2T, eps_t, ident, ones)
            _phase3(nc, tc, x2_dram, h2T, w1, b1_sb, w2, b2_bc, out)

    _split_waits(nc)
    return nc


def _layernorm_tile(nc, pool, xt, eps_t, out_dtype=bf16):
    """token-major LN on a [128, D] tile (w/b folded into weights)."""
    stats = pool.tile([128, 4, 6], f32, tag="lnstats")
    xg = xt.rearrange("p (n f) -> p n f", f=512)
    for i in range(4):
        nc.vector.bn_stats(out=stats[:, i, :], in_=xg[:, i, :])
    mv = pool.tile([128, 2], f32, tag="lnmv")
    nc.vector.bn_aggr(out=mv, in_=stats)
    rs = pool.tile([128, 1], f32, tag="lnrs")
    nc.scalar.activation(out=rs, in_=mv[:, 1:2],
                         func=mybir.ActivationFunctionType.Sqrt,
                         bias=eps_t, scale=1.0)
    nc.vector.reciprocal(out=rs, in_=rs)
    h = pool.tile([128, D], out_dtype, tag="lnh")
    nc.vector.tensor_scalar(out=h, in0=xt, scalar1=mv[:, 0:1], scalar2=rs,
                            op0=mybir.AluOpType.subtract,
                            op1=mybir.AluOpType.mult)
    return h


def _ln_transpose(nc, tc, src_dram, eps_t, ident, hT_all, psT, sb,
                  src_dtype):
    """LN each of 4 [128, D] token tiles of src_dram, then PE-transpose into
    hT_all [128, DT16, 512] (d-major)."""
    for t in range(4):
        xt = sb.tile([128, D], src_dtype, tag="x")
        nc.sync.dma_start(out=xt, in_=src_dram.ap()[t * 128:(t + 1) * 128, :])
        h = _layernorm_tile(nc, sb, xt, eps_t)
        for dg in range(4):
            ps = psT.tile([128, 512], bf16, tag="psT")
            for i in range(4):
                dt = dg * 4 + i
                nc.tensor.transpose(ps[:, i * 128:(i + 1) * 128],
                                    h[:, dt * 128:(dt + 1) * 128], ident)
            nc.scalar.copy(
                out=hT_all[:, dg * 4:(dg + 1) * 4, t * 128:(t + 1) * 128],
                in_=ps.rearrange("p (i f) -> p i f", f=128))


def _phase1(nc, tc, xp, kT_local, v_chunks, kT_full, v_full,
            wk_p, wq_p, wv_p, bq_pp, eps_t, ident, qT, kt_own, v_own):
    """LN1 on own 512 tokens (PE-transposed to hT); per group g: K proj
    (dt-outer so it starts on the first wk tile) -> k-gather -> V proj ->
    v-gather -> Q proj for 4 tiles (wq streamed on the otherwise idle
    vector DGE queue).  K/V outputs stay SBUF-resident for the diagonal
    attention tiles."""
    with (
        tc.tile_pool(name="p1w", bufs=1) as wpool,
        tc.tile_pool(name="p1ht", bufs=1) as htpool,
        tc.tile_pool(name="p1", bufs=2) as sb,
        tc.tile_pool(name="p1q", bufs=2) as qwpool,
        tc.tile_pool(name="p1psT", bufs=2, space="PSUM") as psT,
        tc.tile_pool(name="p1ps", bufs=4, space="PSUM") as psK,
        tc.tile_pool(name="p1psv", bufs=2, space="PSUM") as psV,
    ):
        hT_all = htpool.tile([128, DT16, QT], bf16, tag="hT")
        bq_sb = wpool.tile([128, DT16], f32, tag="bq")
        nc.scalar.dma_start(out=bq_sb, in_=bq_pp.ap())

        # LN + transpose first (x loads lead the sync DMA queue)
        _ln_transpose(nc, tc, xp, eps_t, ident, hT_all, psT, sb, bf16)

        # interleaved per-group K proj -> k-gather -> V proj -> v-gather ->
        # Q proj, so the serial cc stream starts as early as possible and
        # hides under the remaining projections.  Gathered copies are fp8
        # (e3m4).
        for g in range(4):
            # K proj, dt-outer with streamed 128KB weight chunks: the
            # first matmuls only need chunk (g, 0), so the PE starts as
            # soon as one chunk has landed.
            pks = [psK.tile([128, QT], f32, tag="psk", name=f"psk{g}_{m}")
                   for m in range(4)]
            for dt in range(DT16):
                wkc = wpool.tile([128, 512], bf16, tag="wk", bufs=4,
                                 name=f"wk{g}_{dt}")
                nc.scalar.dma_start(out=wkc, in_=wk_p.ap()[g, dt])
                for i, m in enumerate(range(4 * g, 4 * g + 4)):
                    nc.tensor.matmul(pks[i],
                                     wkc[:, i * 128:(i + 1) * 128],
                                     hT_all[:, dt, :], start=(dt == 0),
                                     stop=(dt == DT16 - 1))
            for i, m in enumerate(range(4 * g, 4 * g + 4)):
                nc.scalar.copy(out=kt_own[m], in_=pks[i])
                kt8 = sb.tile([128, QT], f8, tag="kt8")
                nc.scalar.copy(out=kt8, in_=pks[i])
                nc.sync.dma_start(out=kT_local.ap()[m * 128:(m + 1) * 128, :],
                                  in_=kt8)
            nc.gpsimd.collective_compute(
                "AllGather", mybir.AluOpType.bypass,
                replica_groups=GROUPS,
                ins=[kT_local.ap()[g * 512:(g + 1) * 512, :]],
                outs=[kT_full[g].ap()])

            nch = g
            # V proj in two 256-wide feature halves (halves wv SBUF and
            # gives finer DMA pipelining)
            for fh in range(2):
                wv = qwpool.tile([128, DT16, 256], bf16, tag="wv")
                nc.scalar.dma_start(out=wv, in_=wv_p.ap()[nch, fh])
                for t in range(4):
                    ps = psV.tile([128, 256], f32)
                    for dt in range(DT16):
                        nc.tensor.matmul(
                            ps, hT_all[:, dt, t * 128:(t + 1) * 128],
                            wv[:, dt, :], start=(dt == 0),
                            stop=(dt == DT16 - 1))
                    nc.scalar.copy(
                        out=v_own[nch][t][:, fh * 256:(fh + 1) * 256],
                        in_=ps)
                    v8 = sb.tile([128, 256], f8, tag="v8")
                    nc.scalar.copy(out=v8, in_=ps)
                    nc.sync.dma_start(
                        out=v_chunks.ap()[nch, t * 128:(t + 1) * 128,
                                          fh * 256:(fh + 1) * 256],
                        in_=v8)
            nc.gpsimd.collective_compute(
                "AllGather", mybir.AluOpType.bypass,
                replica_groups=GROUPS,
                ins=[v_chunks.ap()[nch]],
                outs=[v_full[nch].ap()])

            # Q projection for this group's 4 feature tiles
            for m in range(4 * g, 4 * g + 4):
                wq = qwpool.tile([128, DT16, 128], bf16, tag="wq")
                nc.gpsimd.dma_start(out=wq, in_=wq_p.ap()[m])
                ps = psK.tile([128, QT], f32, tag="psk")
                for dt in range(DT16):
                    nc.tensor.matmul(ps, wq[:, dt, :], hT_all[:, dt, :],
                                     start=(dt == 0), stop=(dt == DT16 - 1))
                nc.vector.tensor_scalar(out=qT[m], in0=ps,
                                        scalar1=bq_sb[:, m:m + 1],
                                        scalar2=None,
                                        op0=mybir.AluOpType.add)


def _phase2(nc, tc, app, qT, kt_own, v_own, kT_full, v_full, acol_sb,
            arow, earow, wo, xres, x2_dram, x2_sb, h2T, eps_t, ident, ones):
    """attention + o_proj + residual -> x2; LN2+transpose interleaved."""
    with (
        tc.tile_pool(name="p2wo", bufs=2) as wopool,
        tc.tile_pool(name="p2s", bufs=4) as spool,
        tc.tile_pool(name="p2at", bufs=1) as atpool,
        tc.tile_pool(name="p2o", bufs=3) as opool,
        tc.tile_pool(name="p2den", bufs=2) as denpool,
    ):

        attnT = {}
        with (
            tc.tile_pool(name="psS", bufs=4, space="PSUM") as psS,
            tc.tile_pool(name="psAV", bufs=2, space="PSUM") as psAV,
            tc.tile_pool(name="psD", bufs=2, space="PSUM") as psD,
        ):
            ones_row = wopool.tile([1, 128], f32, tag="ones_row")
            nc.vector.memset(ones_row, 1.0)
            pending = []

            def _flush_pending(nc_, pend):
                """Emit the den/av matmuls (and slot finalization) for the
                oldest pipelined stage."""
                if not pend:
                    return
                e = pend.pop(0)
                nc.tensor.matmul(e["pden"], ones, e["es"],
                                 start=e["den_start"], stop=e["den_stop"])
                for i, vt in enumerate(e["vts"]):
                    ai = e["av_base"] + i
                    nc.tensor.matmul(e["pav"], vt,
                                     e["es"][:, i * QW:(i + 1) * QW],
                                     start=(ai == 0),
                                     stop=(ai == e["nav"] - 1))
                if e["fin"] is not None:
                    h_, j_, pav_ = e["fin"]
                    dtmp = denpool.tile([1, 2 * QW], f32, tag="dtmp")
                    nc.scalar.copy(out=dtmp, in_=e["pden"])
                    dsum = denpool.tile([1, QW], f32, tag="dsum")
                    nc.vector.tensor_add(out=dsum, in0=dtmp[:, :QW],
                                         in1=dtmp[:, QW:])
                    rec = denpool.tile([1, QW], f32, tag="rec")
                    # 1/x as exp(-ln(x)) on the scalar engine: keeps the
                    # multi-pass DVE reciprocal (1.7us, head-of-line) off
                    # the stt-critical vector FIFO
                    lnd = denpool.tile([1, QW], f32, tag="lnd")
                    nc.scalar.activation(
                        out=lnd, in_=dsum,
                        func=mybir.ActivationFunctionType.Ln)
                    nc.scalar.activation(
                        out=rec, in_=lnd,
                        func=mybir.ActivationFunctionType.Exp, scale=-1.0)
                    # broadcast rec across partitions via a K=1 matmul
                    # (into the spare half of this slot's avrec bank)
                    recb = e["avrec"][:, QW:]
                    nc.tensor.matmul(recb, ones_row, rec,
                                     start=True, stop=True)
                    avs = denpool.tile([128, QW], bf16, tag="avs")
                    nc.scalar.copy(out=avs, in_=pav_)
                    at = atpool.tile([128, QW], bf16, tag=f"at{h_}_{j_}",
                                     name=f"at{h_}_{j_}")
                    nc.vector.tensor_mul(out=at, in0=recb, in1=avs)
                    attnT[(h_, j_)] = at

            DEPTH = 2   # pending-stage pipeline depth (hides the exp chain)
            for h in range(H):
                g, hh = divmod(h, 4)
                # per-head tiles double-buffered by head parity out of the
                # persistent pool (no WAR against phase1 SBUF regions)
                kt_sb = app.tile([128, 4, QT], f8, tag=f"kt{h % 2}",
                                 name=f"kt_sb{h}")
                for rr in range(4):
                    nc.sync.dma_start(
                        out=kt_sb[:, rr, :],
                        in_=kT_full[g].ap()[rr * 512 + hh * 128:
                                            rr * 512 + (hh + 1) * 128, :])
                # gathered v for this head: [128 tok-part, 16 tok-tile, 128 hd]
                v_sb = app.tile([128, NKT, 128], f8, tag=f"vt{h % 2}",
                                name=f"v_sb{h}")
                nc.sync.dma_start(
                    out=v_sb,
                    in_=v_full[g].ap()[:, hh * 128:(hh + 1) * 128]
                    .rearrange("(n p) f -> p n f", p=128))
                ar_sb = app.tile([128, 2, QW], f32, tag=f"ar{h % 2}",
                                 name=f"ar_sb{h}")
                nc.sync.dma_start(out=ar_sb,
                                  in_=arow.ap()[h].rearrange("v p f -> p v f"))
                er_sb = app.tile([128, 2 * QW], bf16, tag=f"er{h % 2}",
                                 name=f"er_sb{h}")
                nc.sync.dma_start(out=er_sb, in_=earow.ap()[h])

                def src_tiles(j, p):
                    src, _ = READ_MAP[j][p]
                    if src[0] == "loc":
                        kt = kt_own[h][:, src[1]:src[1] + 128]
                        vt = v_own[g][src[1] // 128][:, hh * 128:(hh + 1) * 128]
                    else:
                        _, rr, col0 = src
                        kt = kt_sb[:, rr, col0:col0 + 128]
                        vt = v_sb[:, rr * 4 + col0 // 128, :]
                    return kt, vt

                for j in range(NSLOT):
                    diag_ps, pairs = FAST_PAIRS[j]
                    coff = h * SK + sum(KEXT[:j])
                    avrec = psAV.tile([128, 2 * QW], f32, tag="avrec")
                    pav = avrec[:, :QW]
                    pden = psD.tile([1, 2 * QW], f32)
                    qslice = qT[h][:, j * QW:(j + 1) * QW]
                    stages = [("diag", list(diag_ps))] + \
                             [("fast", list(pr)) for pr in pairs]
                    nst = len(stages)

                    for s, (kind, plist) in enumerate(stages):
                        # score matmuls of this stage (ahead of the pending
                        # den/av so the tensor queue never stalls on exp)
                        ps = psS.tile([128, 2 * QW], f32, tag="ps")
                        for i, p in enumerate(plist):
                            kt, _ = src_tiles(j, p)
                            nc.tensor.matmul(ps[:, i * QW:(i + 1) * QW], kt,
                                             qslice, start=True, stop=True)
                        if len(pending) >= DEPTH:
                            _flush_pending(nc, pending)
                        if kind == "diag":
                            ss = spool.tile([128, 2 * QW], f32, tag="ss")
                            for i, p in enumerate(plist):
                                nc.vector.scalar_tensor_tensor(
                                    out=ss[:, i * QW:(i + 1) * QW],
                                    in0=ps[:, i * QW:(i + 1) * QW],
                                    scalar=acol_sb[:, coff + p:coff + p + 1],
                                    in1=ar_sb[:, i, :],
                                    op0=mybir.AluOpType.add,
                                    op1=mybir.AluOpType.subtract)
                            es = spool.tile([128, 2 * QW], bf16, tag="es")
                            nc.scalar.activation(
                                out=es, in_=ss,
                                func=mybir.ActivationFunctionType.Exp)
                        else:
                            # exp(ps + acol) via the activation bias (per
                            # k-partition column term, exact masking via
                            # exp(-1e9)=0), then one full-width mul by the
                            # row factor exp(-slope*qq).
                            es0 = spool.tile([128, 2 * QW], bf16, tag="es0")
                            for i, p in enumerate(plist):
                                nc.scalar.activation(
                                    out=es0[:, i * QW:(i + 1) * QW],
                                    in_=ps[:, i * QW:(i + 1) * QW],
                                    func=mybir.ActivationFunctionType.Exp,
                                    bias=acol_sb[:, coff + p:coff + p + 1],
                                    scale=1.0)
                            es = spool.tile([128, 2 * QW], bf16, tag="es")
                            nc.vector.tensor_mul(out=es, in0=es0, in1=er_sb)
                        vts = [src_tiles(j, p)[1] for p in plist]
                        fin = None
                        if s == nst - 1:
                            fin = (h, j, pav)
                        pending.append(dict(
                            es=es, vts=vts, pden=pden, pav=pav, avrec=avrec,
                            den_start=(s == 0), den_stop=(s == nst - 1),
                            av_base=2 * s, nav=2 * nst, fin=fin))

            while pending:
                _flush_pending(nc, pending)

        with (
            tc.tile_pool(name="psO", bufs=4, space="PSUM") as psO,
            tc.tile_pool(name="psT2", bufs=2, space="PSUM") as psT2,
        ):
            # o_proj dc-outer with streamed wo chunks (each read once).
            # wo loads go on the gpsimd DGE queue: dma_start is an engine
            # instruction, and on the scalar queue it would block the
            # attention exp chain behind its SBUF-free wait.  After the
            # last dc chunk of a token block: LN2 + PE-transpose, so
            # phase3's MLP can start immediately.
            for dc in range(4):
                wod = wopool.tile([128, DT16, 512], bf16, tag="wod")
                nc.gpsimd.dma_start(out=wod, in_=wo.ap()[dc])
                for j in range(NSLOT):
                    for tt in range(QW // 128):
                        ps = psO.tile([128, 512], f32,
                                      name=f"psO{dc}_{j}_{tt}", tag="psO")
                        for h in range(H):
                            at = attnT[(h, j)][:, tt * 128:(tt + 1) * 128]
                            nc.tensor.matmul(ps, at, wod[:, h, :],
                                             start=(h == 0),
                                             stop=(h == H - 1))
                        row0 = j * QW + tt * 128
                        tslot = j * 2 + tt
                        xr = opool.tile([128, 512], f32, tag="xr")
                        nc.sync.dma_start(
                            out=xr, in_=xres.ap()[row0:row0 + 128,
                                                  dc * 512:(dc + 1) * 512])
                        x2 = opool.tile([128, 512], f32, tag="x2")
                        nc.vector.tensor_add(out=x2, in0=ps, in1=xr)
                        nc.scalar.copy(
                            out=x2_sb[:, tslot, dc * 512:(dc + 1) * 512],
                            in_=x2)
                        nc.sync.dma_start(
                            out=x2_dram.ap()[row0:row0 + 128,
                                             dc * 512:(dc + 1) * 512], in_=x2)
                        if dc == 3:
                            hln = _layernorm_tile(nc, opool,
                                                  x2_sb[:, tslot, :], eps_t)
                            for dg in range(4):
                                pst = psT2.tile([128, 512], bf16, tag="psT2")
                                for i in range(4):
                                    dt = dg * 4 + i
                                    nc.tensor.transpose(
                                        pst[:, i * 128:(i + 1) * 128],
                                        hln[:, dt * 128:(dt + 1) * 128],
                                        ident)
                                nc.scalar.copy(
                                    out=h2T[:, dg * 4:(dg + 1) * 4,
                                            tslot * 128:(tslot + 1) * 128],
                                    in_=pst.rearrange("p (i f) -> p i f",
                                                      f=128))


def _phase3(nc, tc, x2_dram, h2T, w1, b1_sb, w2, b2_bc, out):
    """GELU MLP + residual on the 512 own tokens (LN2 done in phase2)."""
    NQ = 16  # f-tiles per w2 quarter-chunk
    with (
        tc.tile_pool(name="p3h", bufs=1) as hpool,
        tc.tile_pool(name="p3m", bufs=1) as mpool,
        tc.tile_pool(name="p3w1", bufs=3) as w1pool,
        tc.tile_pool(name="p3w2", bufs=2) as w2pool,
        tc.tile_pool(name="p3x2", bufs=2) as x2pool,
        tc.tile_pool(name="psM1", bufs=2, space="PSUM") as psM1,
        tc.tile_pool(name="psM2", bufs=4, space="PSUM") as psM2,
    ):
        b2_sb = hpool.tile([128, D], f32, tag="b2")
        nc.gpsimd.dma_start(out=b2_sb, in_=b2_bc.ap())

        # MLP1 + gelu -> m1^T tiles [128 f, 512].  Weight loads go on the
        # gpsimd DGE queue so the scalar FIFO carries only the gelus.
        m1 = []
        for m in range(FT64):
            w1t = w1pool.tile([128, DT16, 128], bf16, tag="w1")
            nc.gpsimd.dma_start(out=w1t, in_=w1.ap()[m])
            ps = psM1.tile([128, QT], f32)
            for dt in range(DT16):
                nc.tensor.matmul(ps, w1t[:, dt, :], h2T[:, dt, :],
                                 start=(dt == 0), stop=(dt == DT16 - 1))
            mt = mpool.tile([128, QT], bf16, tag=f"m1_{m}")
            nc.scalar.activation(
                out=mt, in_=ps,
                func=mybir.ActivationFunctionType.Gelu_apprx_tanh,
                bias=b1_sb[:, m:m + 1], scale=1.0)
            m1.append(mt)
        # MLP2 (token-major out) + residual + b2; w2 streamed in quarter
        # chunks, 4 psum banks accumulate one t-tile each across quarters.
        for dc in range(4):
            pss = [psM2.tile([128, 512], f32, name=f"psm2_{t}", tag="psm2")
                   for t in range(4)]
            for qc in range(4):
                w2t = w2pool.tile([128, NQ, 512], bf16, tag="w2")
                nc.gpsimd.dma_start(out=w2t, in_=w2.ap()[dc, qc])
                for t in range(4):
                    for f in range(NQ):
                        ft = qc * NQ + f
                        nc.tensor.matmul(
                            pss[t], m1[ft][:, t * 128:(t + 1) * 128],
                            w2t[:, f, :],
                            start=(ft == 0), stop=(ft == FT64 - 1))
            for t in range(4):
                x2t = x2pool.tile([128, 512], f32, tag="x2rd")
                nc.sync.dma_start(
                    out=x2t, in_=x2_dram.ap()[t * 128:(t + 1) * 128,
                                              dc * 512:(dc + 1) * 512])
                s1 = x2pool.tile([128, 512], f32, tag="s1")
                nc.vector.tensor_add(out=s1, in0=pss[t], in1=x2t)
                o = x2pool.tile([128, 512], f32, tag="o")
                nc.vector.tensor_add(out=o, in0=s1,
                                     in1=b2_sb[:, dc * 512:(dc + 1) * 512])
                nc.sync.dma_start(
                    out=out.ap()[t * 128:(t + 1) * 128,
                                 dc * 512:(dc + 1) * 512], in_=o)


# ---------------------------------------------------------------------------
# host wrapper
# ---------------------------------------------------------------------------
_nc_cache = {}


def _get_nc():
    if "nc" not in _nc_cache:
        _nc_cache["nc"] = build_nc()
    return _nc_cache["nc"]


def _own_tokens(r):
    return np.concatenate([np.arange(r * QW, (r + 1) * QW),
                           np.arange((r + 4) * QW, (r + 5) * QW)])


def _prep_inputs(x, ln1_w, ln1_b, wqkv, bqkv, wo, bo, ln2_w, ln2_b,
                 w1, b1, w2, b2):
    slopes = _alibi_slopes(H)
    wqkv_f = (ln1_w[:, None] * wqkv).astype(np.float32)
    bqkv_f = (ln1_b @ wqkv + bqkv).astype(np.float32)
    wqkv_f[:, :D] *= QSCALE
    bqkv_f[:D] *= QSCALE
    w1_f = (ln2_w[:, None] * w1).astype(np.float32)
    b1_f = (ln2_b @ w1 + b1).astype(np.float32)

    wqkv_b = wqkv_f.astype(ml_dtypes.bfloat16)
    # contiguous per-tile repacks (full-bandwidth DMA streams)
    wk_pp = np.ascontiguousarray(
        wqkv_b[:, D:2 * D].reshape(DT16, 128, 4, 512).transpose(2, 0, 1, 3))
    wq_pp = np.ascontiguousarray(
        wqkv_b[:, :D].reshape(DT16, 128, DT16, 128).transpose(2, 1, 0, 3))
    wv_pp = np.ascontiguousarray(
        wqkv_b[:, 2 * D:].reshape(DT16, 128, 4, 2, 256)
        .transpose(2, 3, 1, 0, 4))
    wo_b = np.ascontiguousarray(
        wo.astype(ml_dtypes.bfloat16).reshape(DT16, 128, 4, 512)
        .transpose(2, 1, 0, 3))
    w1_b = w1_f.astype(ml_dtypes.bfloat16)
    w1_pp = np.ascontiguousarray(
        w1_b.reshape(DT16, 128, FT64, 128).transpose(2, 1, 0, 3))
    w2_b = w2.astype(ml_dtypes.bfloat16)
    w2_pp = np.ascontiguousarray(
        w2_b.reshape(4, 16, 128, 4, 512).transpose(3, 0, 2, 1, 4))

    bq_pp = bqkv_f[:D].reshape(DT16, 128).T.copy().astype(np.float32)
    b1_pp = b1_f.reshape(FT64, 128).T.copy().astype(np.float32)
    b2_bc = np.broadcast_to(b2.astype(np.float32), (128, D)).copy()
    # v-bias contributes exactly bv @ wo to the attention output
    res_const = (bo + bqkv_f[2 * D:] @ wo).astype(np.float32)
    arow = _build_arow(slopes)
    earow = _build_earow(slopes)

    in_maps = []
    metas = []
    for c in range(NCORES):
        batch, r = divmod(c, GS)
        tok = _own_tokens(r)
        xp32 = np.ascontiguousarray(x[batch][tok]).astype(np.float32)
        xr = (xp32 + res_const[None, :]).astype(np.float32)
        acol_r = _build_acol(r, slopes)
        in_maps.append({
            "xp": xp32.astype(ml_dtypes.bfloat16), "xres": xr,
            "wk_p": wk_pp, "wq_p": wq_pp, "wv_p": wv_pp, "bq_pp": bq_pp,
            "wo": wo_b, "w1": w1_pp, "b1_pp": b1_pp,
            "w2": w2_pp, "b2_bc": b2_bc,
            "acol": acol_r,
            "arow": arow, "earow": earow,
        })
        metas.append((batch, tok))
    return in_maps, metas


last_result = None


def _install_ntff_hook_shim():
    """Register the boot script's ctypes NTFF hook under the module name
    bass_utils expects, and disable artifact upload (zero-egress box)."""
    import sys as _sys
    import types
    if "antenv.axon_hooks" not in _sys.modules:
        import importlib
        tb = importlib.import_module("trn_agent_boot.trn_boot")
        hook = tb._ntff_profile_via_ctypes("/opt/axon/libaxon_pjrt.so")
        mod = types.ModuleType("antenv.axon_hooks")
        mod.get_axon_ntff_profile_hook = lambda: hook
        _sys.modules["antenv.axon_hooks"] = mod
    import concourse.bass_utils as bu
    bu.upload_artifacts = lambda tmpdir: "(upload disabled)"


_ldw_patched = [False]


def _maybe_enable_ldw_opt():
    """Opt-in experiment: flip walrus --enable-ldw-opt to true."""
    if _ldw_patched[0] or not os.environ.get("KBENCH_LDW_OPT"):
        return
    import concourse.bass_utils as bu
    orig = bu.run_command

    def patched(cmd, *a, **kw):
        cmd = [("--enable-ldw-opt=true" if c == "--enable-ldw-opt=false"
                else c) for c in cmd]
        return orig(cmd, *a, **kw)

    bu.run_command = patched
    _ldw_patched[0] = True


def kernel(**inputs):
    global last_result
    _maybe_enable_ldw_opt()
    args = {k: np.asarray(v, dtype=np.float32) for k, v in inputs.items()}
    in_maps, metas = _prep_inputs(
        args["x"], args["ln1_w"], args["ln1_b"], args["wqkv"], args["bqkv"],
        args["wo"], args["bo"], args["ln2_w"], args["ln2_b"],
        args["w1"], args["b1"], args["w2"], args["b2"])
    nc = _get_nc()
    kwargs = {}
    if os.environ.get("KBENCH_TRACE"):
        _install_ntff_hook_shim()
        kwargs = dict(trace=True,
                      trace_cores=[int(c) for c in
                                   os.environ.get("KBENCH_TRACE_CORES",
                                                  "0").split(",")])
    res = run_bass_kernel_spmd(nc, in_maps, core_ids=list(range(NCORES)),
                               **kwargs)
    last_result = res
    out = np.empty((B, S, D), dtype=np.float32)
    for c in range(NCORES):
        batch, tok = metas[c]
        out[batch, tok] = res.results[c]["out"]
    return out



# revision 42
# speedup vs baseline: 1.1324x; 1.1324x over previous
"""Bloom transformer block on 8 Trainium2 NeuronCores.

Sharding: core c handles batch c//4 and 512 of its 2048 tokens (two
causally-balanced 256-token q-tiles {r, r+4}, r = c%4).  Each core computes
LN1 + Q/K/V projections for ONLY its own 512 tokens; K^T and V slices are
then shared within each 4-core batch group via chunked AllGather collectives
(one per 4-head group, interleaved with compute so the cc stream hides under
projection/attention).  Attention/o_proj/MLP run on own tokens only.  The
diagonal attention k-tiles (own tokens) are read from local SBUF at fixed
program positions, so the triangular causal masks stay compile-time; all
remaining rank-dependence lives in host-prepared acol/eacol data (alibi
columns + masking of invalid gathered tiles).  The device program is
identical across cores (SPMD).  The host concatenates the output slices.

Softmax: scores^T layout [k, q], no max-subtraction (scores are bounded).
Off-diagonal tiles use the factored form
    es = exp(s) * exp(acol) * exp(-slope*qq)
(a per-k-partition column factor applied on gpsimd and a row-tile factor
applied on vector, both in bf16 after a batched scalar-engine exp straight
from PSUM); masked tiles get exp(NEG)=0 through the column factor.
Diagonal tiles keep the exact pre-exp path (vector scalar_tensor_tensor
with the triangular mask row tiles) since their column term is positive
and would overflow the factored exp.

Exact math shortcuts: the k-projection bias adds a per-query constant to
every score row, so it cancels in softmax and is dropped; the v-projection
bias adds exactly bv to each attention output (probs sum to 1), so bv @ wo
is folded into the host-prepared residual.  LN affine params are folded
into the following matmul weights; 1/sqrt(HD) is folded into Wq.
"""

import math
import os

import numpy as np
import ml_dtypes

import concourse.bass as bass
import concourse.tile as tile
from concourse import mybir
from concourse.bass_utils import run_bass_kernel_spmd

B, S, D, H = 2, 2048, 2048, 16
HD = D // H          # 128
FF = 4 * D           # 8192
EPS = 1e-5
NCORES = 8
GS = 4               # cores per batch (group size)
QT = S // GS         # own tokens per core = 512
NEG = -1.0e9
QW = 256             # q-tile width (tokens) in attention
NSLOT = QT // QW     # 2 slots per core
NKT = S // 128       # 16 k-tiles
DT16 = D // 128      # 16 feature tiles of 128
FT64 = FF // 128     # 64 FF tiles
QSCALE = 1.0 / math.sqrt(HD)
GROUPS = [[0, 1, 2, 3], [4, 5, 6, 7]]
# per-slot read extents (in 128-wide k-tiles): 2 local diag + gathered
KEXT = [8, 16]
SK = sum(KEXT)       # 24 acol columns per head

f32 = mybir.dt.float32
bf16 = mybir.dt.bfloat16
f8 = mybir.dt.float8e3


def _alibi_slopes(num_heads):
    closest = 2 ** math.floor(math.log2(num_heads))
    base = 2.0 ** (-(2.0 ** (-(math.log2(closest) - 3))))
    powers = np.arange(1, 1 + closest, dtype=np.float64)
    slopes = base ** powers
    if closest != num_heads:
        extra_base = 2.0 ** (-(2.0 ** (-(math.log2(2 * closest) - 3))))
        num_rem = min(closest, num_heads - closest)
        extra_powers = np.arange(1, 1 + 2 * num_rem, 2, dtype=np.float64)
        slopes = np.concatenate([slopes, extra_base ** extra_powers])
    return slopes.astype(np.float32)


# ---------------------------------------------------------------------------
# wait-split post-pass: this walrus build supports a single sync-wait per
# instruction; excess waits move onto preceding NoOps on the same engine.
# ---------------------------------------------------------------------------
_ctr = [0]


def _split_waits(nc, maxw=1):
    for f in nc.m.functions:
        for bb in f.blocks:
            out = []
            changed = False
            for ins in bb.instructions:
                si = ins.sync_info
                waits = list(si.on_wait) if (si and si.on_wait) else []
                if len(waits) > maxw:
                    head, keep = waits[:-maxw], waits[-maxw:]
                    for w in head:
                        _ctr[0] += 1
                        nop = mybir.InstNoOp(name=f"I-waitsplit-{_ctr[0]}")
                        nop.engine = ins.engine
                        nop.sync_info = mybir.SyncInfo(on_wait=[w], on_update=[])
                        out.append(nop)
                    si.on_wait = keep
                    changed = True
                out.append(ins)
            if changed:
                bb.instructions = out
    return nc


# ---------------------------------------------------------------------------
# attention p-loop read map (program-level, rank-independent)
# entries: ("loc", col0) local kt_own column, or ("rr", rank, col0) gathered
# tile kt_sb[:, rank, col0:col0+128].  var: 0 plain (fast exp path),
# 1/2 diag triangle (exact pre-exp path).
# ---------------------------------------------------------------------------
def _read_map():
    plan = []                     # per slot: list of (src, var)
    s0 = [(("loc", 0), 1), (("loc", 128), 2)]
    for rr in range(3):
        for half in range(2):
            s0.append((("rr", rr, half * 128), 0))
    plan.append(s0)
    s1 = [(("loc", 256), 1), (("loc", 384), 2)]
    for rr in range(4):
        for half in range(2):
            s1.append((("rr", rr, half * 128), 0))
    for rr in range(3):
        for half in range(2):
            s1.append((("rr", rr, 256 + half * 128), 0))
    plan.append(s1)
    return plan


READ_MAP = _read_map()
# fast-path (var0) positions per slot, grouped in pairs for batched exp
FAST_PAIRS = []
for _j in range(NSLOT):
    _fp = [p for p, (_s, _v) in enumerate(READ_MAP[_j]) if _v != 0]
    _fa = [p for p, (_s, _v) in enumerate(READ_MAP[_j]) if _v == 0]
    FAST_PAIRS.append((_fp, [(_fa[i], _fa[i + 1])
                             for i in range(0, len(_fa), 2)]))


def _build_acol(r, slopes):
    """Per-core [128, H*SK] f32 column tiles: value slope_h*(k_orig - q_base)
    per in-tile k position, or NEG for masked (beyond-causal or
    locally-handled) gathered tiles."""
    out = np.empty((128, H * SK), dtype=np.float32)
    kk = np.arange(128, dtype=np.float64)
    for h in range(H):
        for j in range(NSLOT):
            qtile = r + 4 * j               # orig 256-q-tile index
            qbase = qtile * QW
            off = h * SK + sum(KEXT[:j])
            for p, (src, var) in enumerate(READ_MAP[j]):
                if src[0] == "loc":
                    # own diag 128-k-tile: src[1] is the kt_own column
                    # offset; in-slot k offset is src[1] % 256
                    out[:, off + p] = slopes[h] * (src[1] % 256 + kk)
                else:
                    _, rr, col0 = src
                    # gathered tile: rank rr's token col0 (0:256 -> orig
                    # 256-tile rr; 256:512 -> orig rr+4)
                    otile = rr if col0 < 256 else rr + 4
                    k0 = otile * QW + (col0 % 256)
                    # active iff strictly before own q-tile (diagonal is
                    # handled locally; beyond-causal masked)
                    if otile < qtile:
                        out[:, off + p] = slopes[h] * (k0 + kk - qbase)
                    else:
                        out[:, off + p] = NEG
    return out


def _build_arow(slopes):
    """Shared [H, 2, 128, QW] f32 row tiles for the diagonal (exact) path:
    var 1/2 = slope_h*qq + 1e9 on the causally-masked cells of the two
    own-diagonal k-tiles (kk > qq, 128+kk > qq)."""
    out = np.empty((H, 2, 128, QW), dtype=np.float32)
    kk = np.arange(128)
    qq = np.arange(QW)
    m0 = (kk[:, None] > qq[None, :]).astype(np.float32) * (-NEG)
    m1 = ((kk[:, None] + 128) > qq[None, :]).astype(np.float32) * (-NEG)
    for h in range(H):
        plain = np.broadcast_to(slopes[h] * qq[None, :].astype(np.float32),
                                (128, QW))
        out[h, 0] = plain + m0
        out[h, 1] = plain + m1
    return out


def _build_earow(slopes):
    """Shared [H, 128, 2*QW] bf16: exp(-slope_h*qq) tiled twice along free
    (row factor for a batch-2 fast-path es tile)."""
    qq = np.arange(QW, dtype=np.float64)
    out = np.empty((H, 128, 2 * QW), dtype=np.float64)
    for h in range(H):
        e = np.exp(-float(slopes[h]) * qq)
        out[h, :, :QW] = e[None, :]
        out[h, :, QW:] = e[None, :]
    return out.astype(ml_dtypes.bfloat16)


# ---------------------------------------------------------------------------
# device program (identical for all cores)
# ---------------------------------------------------------------------------
def build_nc():
    nc = bass.Bass(target_bir_lowering=False)

    xp = nc.dram_tensor("xp", [QT, D], bf16, kind="ExternalInput")
    xres = nc.dram_tensor("xres", [QT, D], f32, kind="ExternalInput")
    wk_p = nc.dram_tensor("wk_p", [4, DT16, 128, 512], bf16,
                          kind="ExternalInput")
    wq_p = nc.dram_tensor("wq_p", [DT16, 128, DT16, 128], bf16,
                          kind="ExternalInput")
    wv_p = nc.dram_tensor("wv_p", [4, 2, 128, DT16, 256], bf16,
                          kind="ExternalInput")
    bq_pp = nc.dram_tensor("bq_pp", [128, DT16], f32, kind="ExternalInput")
    wo = nc.dram_tensor("wo", [4, 128, DT16, 512], bf16, kind="ExternalInput")
    w1 = nc.dram_tensor("w1", [FT64, 128, DT16, 128], bf16,
                        kind="ExternalInput")
    b1_pp = nc.dram_tensor("b1_pp", [128, FT64], f32, kind="ExternalInput")
    w2 = nc.dram_tensor("w2", [4, 4, 128, 16, 512], bf16,
                        kind="ExternalInput")
    b2_bc = nc.dram_tensor("b2_bc", [128, D], f32, kind="ExternalInput")
    acol = nc.dram_tensor("acol", [128, H * SK], f32, kind="ExternalInput")
    arow = nc.dram_tensor("arow", [H, 2, 128, QW], f32, kind="ExternalInput")
    earow = nc.dram_tensor("earow", [H, 128, 2 * QW], bf16,
                           kind="ExternalInput")

    out = nc.dram_tensor("out", [QT, D], f32, kind="ExternalOutput")

    # per-group staging tensors: separate DRAM tensors so group g+1's
    # stores carry no (whole-tensor WAR) dependency on gather g's reads
    kT_local = [nc.dram_tensor(f"kT_local{g}", [512, QT], f8, kind="Internal")
                for g in range(4)]
    v_chunks = [nc.dram_tensor(f"v_chunks{g}", [QT, 512], f8, kind="Internal")
                for g in range(4)]
    kT_full = [nc.dram_tensor(f"kT_full{g}", [4 * 512, QT], f8,
                              kind="Internal") for g in range(4)]
    v_full = [nc.dram_tensor(f"v_full{g}", [4 * QT, 512], f8,
                             kind="Internal") for g in range(4)]
    x2_dram = nc.dram_tensor("x2_dram", [QT, D], f32, kind="Internal")
    rec_dram = nc.dram_tensor("rec_dram", [NSLOT * H, QW], f32, kind="Internal")
    ident_dram = nc.inline_tensor(np.eye(128, dtype=ml_dtypes.bfloat16),
                                  name="ident128")

    with tile.TileContext(nc) as tc:
        with tc.tile_pool(name="persist", bufs=1) as pp:
            ones = pp.tile([128, 1], bf16, tag="ones")
            nc.vector.memset(ones, 1.0)
            eps_t = pp.tile([128, 1], f32, tag="eps")
            nc.vector.memset(eps_t, EPS)
            ident = pp.tile([128, 128], bf16, tag="ident")
            nc.sync.dma_start(out=ident, in_=ident_dram.ap())
            b1_sb = pp.tile([128, FT64], f32, tag="b1")
            nc.scalar.dma_start(out=b1_sb, in_=b1_pp.ap())
            # bf16 mirror of x2 for LN2 (avoids the DRAM roundtrip before
            # phase3); the f32 residual still comes from x2_dram.
            x2_sb = pp.tile([128, 4, D], bf16, tag="x2sb")
            # LN2 output (transposed) — produced inside phase2 so it
            # overlaps o_proj; consumed by MLP1 in phase3.
            h2T = pp.tile([128, DT16, QT], bf16, tag="h2T")

            with tc.tile_pool(name="attn_persist", bufs=1) as app:
                qT = [app.tile([128, QT], bf16, tag=f"qT{m}", name=f"qT{m}")
                      for m in range(DT16)]
                kt_own = [app.tile([128, QT], bf16, tag=f"ktown{m}",
                                   name=f"ktown{m}") for m in range(DT16)]
                # v_own[nch][t]: [128 tok, 512 feat]
                v_own = [[app.tile([128, 512], bf16, tag=f"vown{n}_{t}",
                                   name=f"vown{n}_{t}") for t in range(4)]
                         for n in range(4)]
                # attention tables live in the persistent pool so their DMAs
                # don't wait on phase1 SBUF frees (loaded at kernel start).
                acol_sb = app.tile([128, H * SK], f32, tag="acol")
                nc.sync.dma_start(out=acol_sb, in_=acol.ap())

                _phase1(nc, tc, xp, kT_local, v_chunks, kT_full,
                        v_full, wk_p, wq_p, wv_p, bq_pp, eps_t, ident, qT,
                        kt_own, v_own)
                _phase2(nc, tc, app, qT, kt_own, v_own, kT_full, v_full,
                        acol_sb, arow, earow, wo, xres, x2_dram,
                        x2_sb, h2T, eps_t, ident, ones)
            _phase3(nc, tc, x2_dram, h2T, w1, b1_sb, w2, b2_bc, out)

    _split_waits(nc)
    return nc


def _layernorm_tile(nc, pool, xt, eps_t, out_dtype=bf16):
    """token-major LN on a [128, D] tile (w/b folded into weights)."""
    stats = pool.tile([128, 4, 6], f32, tag="lnstats")
    xg = xt.rearrange("p (n f) -> p n f", f=512)
    for i in range(4):
        nc.vector.bn_stats(out=stats[:, i, :], in_=xg[:, i, :])
    mv = pool.tile([128, 2], f32, tag="lnmv")
    nc.vector.bn_aggr(out=mv, in_=stats)
    rs = pool.tile([128, 1], f32, tag="lnrs")
    nc.scalar.activation(out=rs, in_=mv[:, 1:2],
                         func=mybir.ActivationFunctionType.Sqrt,
                         bias=eps_t, scale=1.0)
    nc.vector.reciprocal(out=rs, in_=rs)
    h = pool.tile([128, D], out_dtype, tag="lnh")
    nc.vector.tensor_scalar(out=h, in0=xt, scalar1=mv[:, 0:1], scalar2=rs,
                            op0=mybir.AluOpType.subtract,
                            op1=mybir.AluOpType.mult)
    return h


def _ln_transpose(nc, tc, src_dram, eps_t, ident, hT_all, psT, sb,
                  src_dtype):
    """LN each of 4 [128, D] token tiles of src_dram, then PE-transpose into
    hT_all [128, DT16, 512] (d-major)."""
    for t in range(4):
        xt = sb.tile([128, D], src_dtype, tag="x")
        nc.sync.dma_start(out=xt, in_=src_dram.ap()[t * 128:(t + 1) * 128, :])
        h = _layernorm_tile(nc, sb, xt, eps_t)
        for dg in range(4):
            ps = psT.tile([128, 512], bf16, tag="psT")
            for i in range(4):
                dt = dg * 4 + i
                nc.tensor.transpose(ps[:, i * 128:(i + 1) * 128],
                                    h[:, dt * 128:(dt + 1) * 128], ident)
            nc.scalar.copy(
                out=hT_all[:, dg * 4:(dg + 1) * 4, t * 128:(t + 1) * 128],
                in_=ps.rearrange("p (i f) -> p i f", f=128))


def _phase1(nc, tc, xp, kT_local, v_chunks, kT_full, v_full,
            wk_p, wq_p, wv_p, bq_pp, eps_t, ident, qT, kt_own, v_own):
    """LN1 on own 512 tokens (PE-transposed to hT); per group g: K proj
    (dt-outer so it starts on the first wk tile) -> k-gather -> V proj ->
    v-gather -> Q proj for 4 tiles (wq streamed on the otherwise idle
    vector DGE queue).  K/V outputs stay SBUF-resident for the diagonal
    attention tiles."""
    with (
        tc.tile_pool(name="p1w", bufs=1) as wpool,
        tc.tile_pool(name="p1ht", bufs=1) as htpool,
        tc.tile_pool(name="p1", bufs=2) as sb,
        tc.tile_pool(name="p1q", bufs=2) as qwpool,
        tc.tile_pool(name="p1psT", bufs=2, space="PSUM") as psT,
        tc.tile_pool(name="p1ps", bufs=4, space="PSUM") as psK,
        tc.tile_pool(name="p1psv", bufs=2, space="PSUM") as psV,
    ):
        hT_all = htpool.tile([128, DT16, QT], bf16, tag="hT")
        bq_sb = wpool.tile([128, DT16], f32, tag="bq")
        nc.scalar.dma_start(out=bq_sb, in_=bq_pp.ap())

        # LN + transpose first (x loads lead the sync DMA queue)
        _ln_transpose(nc, tc, xp, eps_t, ident, hT_all, psT, sb, bf16)

        # interleaved per-group K proj -> k-gather -> V proj -> v-gather ->
        # Q proj, so the serial cc stream starts as early as possible and
        # hides under the remaining projections.  Gathered copies are fp8
        # (e3m4).
        for g in range(4):
            # K proj, dt-outer with streamed 128KB weight chunks: the
            # first matmuls only need chunk (g, 0), so the PE starts as
            # soon as one chunk has landed.
            pks = [psK.tile([128, QT], f32, tag="psk", name=f"psk{g}_{m}")
                   for m in range(4)]
            for dt in range(DT16):
                wkc = wpool.tile([128, 512], bf16, tag="wk", bufs=4,
                                 name=f"wk{g}_{dt}")
                nc.scalar.dma_start(out=wkc, in_=wk_p.ap()[g, dt])
                for i, m in enumerate(range(4 * g, 4 * g + 4)):
                    nc.tensor.matmul(pks[i],
                                     wkc[:, i * 128:(i + 1) * 128],
                                     hT_all[:, dt, :], start=(dt == 0),
                                     stop=(dt == DT16 - 1))
            for i, m in enumerate(range(4 * g, 4 * g + 4)):
                nc.scalar.copy(out=kt_own[m], in_=pks[i])
                kt8 = sb.tile([128, QT], f8, tag="kt8", bufs=8)
                nc.scalar.copy(out=kt8, in_=pks[i])
                nc.sync.dma_start(out=kT_local[g].ap()[i * 128:(i + 1) * 128,
                                                       :],
                                  in_=kt8)
            nc.gpsimd.collective_compute(
                "AllGather", mybir.AluOpType.bypass,
                replica_groups=GROUPS,
                ins=[kT_local[g].ap()],
                outs=[kT_full[g].ap()])

            nch = g
            # V proj in two 256-wide feature halves (halves wv SBUF and
            # gives finer DMA pipelining)
            for fh in range(2):
                wv = qwpool.tile([128, DT16, 256], bf16, tag="wv")
                nc.scalar.dma_start(out=wv, in_=wv_p.ap()[nch, fh])
                for t in range(4):
                    ps = psV.tile([128, 256], f32)
                    for dt in range(DT16):
                        nc.tensor.matmul(
                            ps, hT_all[:, dt, t * 128:(t + 1) * 128],
                            wv[:, dt, :], start=(dt == 0),
                            stop=(dt == DT16 - 1))
                    nc.scalar.copy(
                        out=v_own[nch][t][:, fh * 256:(fh + 1) * 256],
                        in_=ps)
                    v8 = sb.tile([128, 256], f8, tag="v8", bufs=8)
                    nc.scalar.copy(out=v8, in_=ps)
                    nc.sync.dma_start(
                        out=v_chunks[nch].ap()[t * 128:(t + 1) * 128,
                                               fh * 256:(fh + 1) * 256],
                        in_=v8)
            nc.gpsimd.collective_compute(
                "AllGather", mybir.AluOpType.bypass,
                replica_groups=GROUPS,
                ins=[v_chunks[nch].ap()],
                outs=[v_full[nch].ap()])

            # Q projection for this group's 4 feature tiles
            for m in range(4 * g, 4 * g + 4):
                wq = qwpool.tile([128, DT16, 128], bf16, tag="wq")
                nc.gpsimd.dma_start(out=wq, in_=wq_p.ap()[m])
                ps = psK.tile([128, QT], f32, tag="psk")
                for dt in range(DT16):
                    nc.tensor.matmul(ps, wq[:, dt, :], hT_all[:, dt, :],
                                     start=(dt == 0), stop=(dt == DT16 - 1))
                nc.vector.tensor_scalar(out=qT[m], in0=ps,
                                        scalar1=bq_sb[:, m:m + 1],
                                        scalar2=None,
                                        op0=mybir.AluOpType.add)


def _phase2(nc, tc, app, qT, kt_own, v_own, kT_full, v_full, acol_sb,
            arow, earow, wo, xres, x2_dram, x2_sb, h2T, eps_t, ident, ones):
    """attention + o_proj + residual -> x2; LN2+transpose interleaved."""
    with (
        tc.tile_pool(name="p2wo", bufs=2) as wopool,
        tc.tile_pool(name="p2s", bufs=4) as spool,
        tc.tile_pool(name="p2at", bufs=1) as atpool,
        tc.tile_pool(name="p2o", bufs=3) as opool,
        tc.tile_pool(name="p2den", bufs=2) as denpool,
    ):

        attnT = {}
        with (
            tc.tile_pool(name="psS", bufs=4, space="PSUM") as psS,
            tc.tile_pool(name="psAV", bufs=2, space="PSUM") as psAV,
            tc.tile_pool(name="psD", bufs=2, space="PSUM") as psD,
        ):
            ones_row = wopool.tile([1, 128], f32, tag="ones_row")
            nc.vector.memset(ones_row, 1.0)
            pending = []

            def _flush_pending(nc_, pend):
                """Emit the den/av matmuls (and slot finalization) for the
                oldest pipelined stage."""
                if not pend:
                    return
                e = pend.pop(0)
                nc.tensor.matmul(e["pden"], ones, e["es"],
                                 start=e["den_start"], stop=e["den_stop"])
                for i, vt in enumerate(e["vts"]):
                    ai = e["av_base"] + i
                    nc.tensor.matmul(e["pav"], vt,
                                     e["es"][:, i * QW:(i + 1) * QW],
                                     start=(ai == 0),
                                     stop=(ai == e["nav"] - 1))
                if e["fin"] is not None:
                    h_, j_, pav_ = e["fin"]
                    dtmp = denpool.tile([1, 2 * QW], f32, tag="dtmp")
                    nc.scalar.copy(out=dtmp, in_=e["pden"])
                    dsum = denpool.tile([1, QW], f32, tag="dsum")
                    nc.vector.tensor_add(out=dsum, in0=dtmp[:, :QW],
                                         in1=dtmp[:, QW:])
                    rec = denpool.tile([1, QW], f32, tag="rec")
                    # 1/x as exp(-ln(x)) on the scalar engine: keeps the
                    # multi-pass DVE reciprocal (1.7us, head-of-line) off
                    # the stt-critical vector FIFO
                    lnd = denpool.tile([1, QW], f32, tag="lnd")
                    nc.scalar.activation(
                        out=lnd, in_=dsum,
                        func=mybir.ActivationFunctionType.Ln)
                    nc.scalar.activation(
                        out=rec, in_=lnd,
                        func=mybir.ActivationFunctionType.Exp, scale=-1.0)
                    # broadcast rec across partitions via a K=1 matmul
                    # (into the spare half of this slot's avrec bank)
                    recb = e["avrec"][:, QW:]
                    nc.tensor.matmul(recb, ones_row, rec,
                                     start=True, stop=True)
                    avs = denpool.tile([128, QW], bf16, tag="avs")
                    nc.scalar.copy(out=avs, in_=pav_)
                    at = atpool.tile([128, QW], bf16, tag=f"at{h_}_{j_}",
                                     name=f"at{h_}_{j_}")
                    nc.vector.tensor_mul(out=at, in0=recb, in1=avs)
                    attnT[(h_, j_)] = at

            DEPTH = 2   # pending-stage pipeline depth (hides the exp chain)
            for h in range(H):
                g, hh = divmod(h, 4)
                # per-head tiles double-buffered by head parity out of the
                # persistent pool (no WAR against phase1 SBUF regions)
                kt_sb = app.tile([128, 4, QT], f8, tag=f"kt{h % 2}",
                                 name=f"kt_sb{h}")
                for rr in range(4):
                    nc.sync.dma_start(
                        out=kt_sb[:, rr, :],
                        in_=kT_full[g].ap()[rr * 512 + hh * 128:
                                            rr * 512 + (hh + 1) * 128, :])
                # gathered v for this head: [128 tok-part, 16 tok-tile, 128 hd]
                v_sb = app.tile([128, NKT, 128], f8, tag=f"vt{h % 2}",
                                name=f"v_sb{h}")
                nc.sync.dma_start(
                    out=v_sb,
                    in_=v_full[g].ap()[:, hh * 128:(hh + 1) * 128]
                    .rearrange("(n p) f -> p n f", p=128))
                ar_sb = app.tile([128, 2, QW], f32, tag=f"ar{h % 2}",
                                 name=f"ar_sb{h}")
                nc.sync.dma_start(out=ar_sb,
                                  in_=arow.ap()[h].rearrange("v p f -> p v f"))
                er_sb = app.tile([128, 2 * QW], bf16, tag=f"er{h % 2}",
                                 name=f"er_sb{h}")
                nc.sync.dma_start(out=er_sb, in_=earow.ap()[h])

                def src_tiles(j, p):
                    src, _ = READ_MAP[j][p]
                    if src[0] == "loc":
                        kt = kt_own[h][:, src[1]:src[1] + 128]
                        vt = v_own[g][src[1] // 128][:, hh * 128:(hh + 1) * 128]
                    else:
                        _, rr, col0 = src
                        kt = kt_sb[:, rr, col0:col0 + 128]
                        vt = v_sb[:, rr * 4 + col0 // 128, :]
                    return kt, vt

                for j in range(NSLOT):
                    diag_ps, pairs = FAST_PAIRS[j]
                    coff = h * SK + sum(KEXT[:j])
                    avrec = psAV.tile([128, 2 * QW], f32, tag="avrec")
                    pav = avrec[:, :QW]
                    pden = psD.tile([1, 2 * QW], f32)
                    qslice = qT[h][:, j * QW:(j + 1) * QW]
                    stages = [("diag", list(diag_ps))] + \
                             [("fast", list(pr)) for pr in pairs]
                    nst = len(stages)

                    for s, (kind, plist) in enumerate(stages):
                        # score matmuls of this stage (ahead of the pending
                        # den/av so the tensor queue never stalls on exp)
                        ps = psS.tile([128, 2 * QW], f32, tag="ps")
                        for i, p in enumerate(plist):
                            kt, _ = src_tiles(j, p)
                            nc.tensor.matmul(ps[:, i * QW:(i + 1) * QW], kt,
                                             qslice, start=True, stop=True)
                        if len(pending) >= DEPTH:
                            _flush_pending(nc, pending)
                        if kind == "diag":
                            ss = spool.tile([128, 2 * QW], f32, tag="ss")
                            for i, p in enumerate(plist):
                                nc.vector.scalar_tensor_tensor(
                                    out=ss[:, i * QW:(i + 1) * QW],
                                    in0=ps[:, i * QW:(i + 1) * QW],
                                    scalar=acol_sb[:, coff + p:coff + p + 1],
                                    in1=ar_sb[:, i, :],
                                    op0=mybir.AluOpType.add,
                                    op1=mybir.AluOpType.subtract)
                            es = spool.tile([128, 2 * QW], bf16, tag="es")
                            nc.scalar.activation(
                                out=es, in_=ss,
                                func=mybir.ActivationFunctionType.Exp)
                        else:
                            # exp(ps + acol) via the activation bias (per
                            # k-partition column term, exact masking via
                            # exp(-1e9)=0), then one full-width mul by the
                            # row factor exp(-slope*qq).
                            es0 = spool.tile([128, 2 * QW], bf16, tag="es0")
                            for i, p in enumerate(plist):
                                nc.scalar.activation(
                                    out=es0[:, i * QW:(i + 1) * QW],
                                    in_=ps[:, i * QW:(i + 1) * QW],
                                    func=mybir.ActivationFunctionType.Exp,
                                    bias=acol_sb[:, coff + p:coff + p + 1],
                                    scale=1.0)
                            es = spool.tile([128, 2 * QW], bf16, tag="es")
                            nc.vector.tensor_mul(out=es, in0=es0, in1=er_sb)
                        vts = [src_tiles(j, p)[1] for p in plist]
                        fin = None
                        if s == nst - 1:
                            fin = (h, j, pav)
                        pending.append(dict(
                            es=es, vts=vts, pden=pden, pav=pav, avrec=avrec,
                            den_start=(s == 0), den_stop=(s == nst - 1),
                            av_base=2 * s, nav=2 * nst, fin=fin))

            while pending:
                _flush_pending(nc, pending)

        with (
            tc.tile_pool(name="psO", bufs=4, space="PSUM") as psO,
            tc.tile_pool(name="psT2", bufs=2, space="PSUM") as psT2,
        ):
            # o_proj dc-outer with streamed wo chunks (each read once).
            # wo loads go on the gpsimd DGE queue: dma_start is an engine
            # instruction, and on the scalar queue it would block the
            # attention exp chain behind its SBUF-free wait.  After the
            # last dc chunk of a token block: LN2 + PE-transpose, so
            # phase3's MLP can start immediately.
            for dc in range(4):
                wod = wopool.tile([128, DT16, 512], bf16, tag="wod")
                nc.gpsimd.dma_start(out=wod, in_=wo.ap()[dc])
                for j in range(NSLOT):
                    for tt in range(QW // 128):
                        ps = psO.tile([128, 512], f32,
                                      name=f"psO{dc}_{j}_{tt}", tag="psO")
                        for h in range(H):
                            at = attnT[(h, j)][:, tt * 128:(tt + 1) * 128]
                            nc.tensor.matmul(ps, at, wod[:, h, :],
                                             start=(h == 0),
                                             stop=(h == H - 1))
                        row0 = j * QW + tt * 128
                        tslot = j * 2 + tt
                        xr = opool.tile([128, 512], f32, tag="xr")
                        nc.sync.dma_start(
                            out=xr, in_=xres.ap()[row0:row0 + 128,
                                                  dc * 512:(dc + 1) * 512])
                        x2 = opool.tile([128, 512], f32, tag="x2")
                        nc.vector.tensor_add(out=x2, in0=ps, in1=xr)
                        nc.scalar.copy(
                            out=x2_sb[:, tslot, dc * 512:(dc + 1) * 512],
                            in_=x2)
                        nc.sync.dma_start(
                            out=x2_dram.ap()[row0:row0 + 128,
                                             dc * 512:(dc + 1) * 512], in_=x2)
                        if dc == 3:
                            hln = _layernorm_tile(nc, opool,
                                                  x2_sb[:, tslot, :], eps_t)
                            for dg in range(4):
                                pst = psT2.tile([128, 512], bf16, tag="psT2")
                                for i in range(4):
                                    dt = dg * 4 + i
                                    nc.tensor.transpose(
                                        pst[:, i * 128:(i + 1) * 128],
                                        hln[:, dt * 128:(dt + 1) * 128],
                                        ident)
                                nc.scalar.copy(
                                    out=h2T[:, dg * 4:(dg + 1) * 4,
                                            tslot * 128:(tslot + 1) * 128],
                                    in_=pst.rearrange("p (i f) -> p i f",
                                                      f=128))


def _phase3(nc, tc, x2_dram, h2T, w1, b1_sb, w2, b2_bc, out):
    """GELU MLP + residual on the 512 own tokens (LN2 done in phase2)."""
    NQ = 16  # f-tiles per w2 quarter-chunk
    with (
        tc.tile_pool(name="p3h", bufs=1) as hpool,
        tc.tile_pool(name="p3m", bufs=1) as mpool,
        tc.tile_pool(name="p3w1", bufs=3) as w1pool,
        tc.tile_pool(name="p3w2", bufs=2) as w2pool,
        tc.tile_pool(name="p3x2", bufs=2) as x2pool,
        tc.tile_pool(name="psM1", bufs=2, space="PSUM") as psM1,
        tc.tile_pool(name="psM2", bufs=4, space="PSUM") as psM2,
    ):
        b2_sb = hpool.tile([128, D], f32, tag="b2")
        nc.scalar.dma_start(out=b2_sb, in_=b2_bc.ap())

        # MLP1 + gelu -> m1^T tiles [128 f, 512].  Weight loads go on the
        # gpsimd DGE queue so the scalar FIFO carries only the gelus.
        m1 = []
        for m in range(FT64):
            w1t = w1pool.tile([128, DT16, 128], bf16, tag="w1")
            nc.scalar.dma_start(out=w1t, in_=w1.ap()[m])
            ps = psM1.tile([128, QT], f32)
            for dt in range(DT16):
                nc.tensor.matmul(ps, w1t[:, dt, :], h2T[:, dt, :],
                                 start=(dt == 0), stop=(dt == DT16 - 1))
            mt = mpool.tile([128, QT], bf16, tag=f"m1_{m}")
            nc.scalar.activation(
                out=mt, in_=ps,
                func=mybir.ActivationFunctionType.Gelu_apprx_tanh,
                bias=b1_sb[:, m:m + 1], scale=1.0)
            m1.append(mt)
        # MLP2 (token-major out) + residual + b2; w2 streamed in quarter
        # chunks, 4 psum banks accumulate one t-tile each across quarters.
        for dc in range(4):
            pss = [psM2.tile([128, 512], f32, name=f"psm2_{t}", tag="psm2")
                   for t in range(4)]
            for qc in range(4):
                w2t = w2pool.tile([128, NQ, 512], bf16, tag="w2")
                nc.scalar.dma_start(out=w2t, in_=w2.ap()[dc, qc])
                for t in range(4):
                    for f in range(NQ):
                        ft = qc * NQ + f
                        nc.tensor.matmul(
                            pss[t], m1[ft][:, t * 128:(t + 1) * 128],
                            w2t[:, f, :],
                            start=(ft == 0), stop=(ft == FT64 - 1))
            for t in range(4):
                x2t = x2pool.tile([128, 512], f32, tag="x2rd")
                nc.sync.dma_start(
                    out=x2t, in_=x2_dram.ap()[t * 128:(t + 1) * 128,
                                              dc * 512:(dc + 1) * 512])
                s1 = x2pool.tile([128, 512], f32, tag="s1")
                nc.vector.tensor_add(out=s1, in0=pss[t], in1=x2t)
                o = x2pool.tile([128, 512], f32, tag="o")
                nc.vector.tensor_add(out=o, in0=s1,
                                     in1=b2_sb[:, dc * 512:(dc + 1) * 512])
                nc.sync.dma_start(
                    out=out.ap()[t * 128:(t + 1) * 128,
                                 dc * 512:(dc + 1) * 512], in_=o)


# ---------------------------------------------------------------------------
# host wrapper
# ---------------------------------------------------------------------------
_nc_cache = {}


def _get_nc():
    if "nc" not in _nc_cache:
        _nc_cache["nc"] = build_nc()
    return _nc_cache["nc"]


def _own_tokens(r):
    return np.concatenate([np.arange(r * QW, (r + 1) * QW),
                           np.arange((r + 4) * QW, (r + 5) * QW)])


def _prep_inputs(x, ln1_w, ln1_b, wqkv, bqkv, wo, bo, ln2_w, ln2_b,
                 w1, b1, w2, b2):
    slopes = _alibi_slopes(H)
    wqkv_f = (ln1_w[:, None] * wqkv).astype(np.float32)
    bqkv_f = (ln1_b @ wqkv + bqkv).astype(np.float32)
    wqkv_f[:, :D] *= QSCALE
    bqkv_f[:D] *= QSCALE
    w1_f = (ln2_w[:, None] * w1).astype(np.float32)
    b1_f = (ln2_b @ w1 + b1).astype(np.float32)

    wqkv_b = wqkv_f.astype(ml_dtypes.bfloat16)
    # contiguous per-tile repacks (full-bandwidth DMA streams)
    wk_pp = np.ascontiguousarray(
        wqkv_b[:, D:2 * D].reshape(DT16, 128, 4, 512).transpose(2, 0, 1, 3))
    wq_pp = np.ascontiguousarray(
        wqkv_b[:, :D].reshape(DT16, 128, DT16, 128).transpose(2, 1, 0, 3))
    wv_pp = np.ascontiguousarray(
        wqkv_b[:, 2 * D:].reshape(DT16, 128, 4, 2, 256)
        .transpose(2, 3, 1, 0, 4))
    wo_b = np.ascontiguousarray(
        wo.astype(ml_dtypes.bfloat16).reshape(DT16, 128, 4, 512)
        .transpose(2, 1, 0, 3))
    w1_b = w1_f.astype(ml_dtypes.bfloat16)
    w1_pp = np.ascontiguousarray(
        w1_b.reshape(DT16, 128, FT64, 128).transpose(2, 1, 0, 3))
    w2_b = w2.astype(ml_dtypes.bfloat16)
    w2_pp = np.ascontiguousarray(
        w2_b.reshape(4, 16, 128, 4, 512).transpose(3, 0, 2, 1, 4))

    bq_pp = bqkv_f[:D].reshape(DT16, 128).T.copy().astype(np.float32)
    b1_pp = b1_f.reshape(FT64, 128).T.copy().astype(np.float32)
    b2_bc = np.broadcast_to(b2.astype(np.float32), (128, D)).copy()
    # v-bias contributes exactly bv @ wo to the attention output
    res_const = (bo + bqkv_f[2 * D:] @ wo).astype(np.float32)
    arow = _build_arow(slopes)
    earow = _build_earow(slopes)

    in_maps = []
    metas = []
    for c in range(NCORES):
        batch, r = divmod(c, GS)
        tok = _own_tokens(r)
        xp32 = np.ascontiguousarray(x[batch][tok]).astype(np.float32)
        xr = (xp32 + res_const[None, :]).astype(np.float32)
        acol_r = _build_acol(r, slopes)
        in_maps.append({
            "xp": xp32.astype(ml_dtypes.bfloat16), "xres": xr,
            "wk_p": wk_pp, "wq_p": wq_pp, "wv_p": wv_pp, "bq_pp": bq_pp,
            "wo": wo_b, "w1": w1_pp, "b1_pp": b1_pp,
            "w2": w2_pp, "b2_bc": b2_bc,
            "acol": acol_r,
            "arow": arow, "earow": earow,
        })
        metas.append((batch, tok))
    return in_maps, metas


last_result = None


def _install_ntff_hook_shim():
    """Register the boot script's ctypes NTFF hook under the module name
    bass_utils expects, and disable artifact upload (zero-egress box)."""
    import sys as _sys
    import types
    if "antenv.axon_hooks" not in _sys.modules:
        import importlib
        tb = importlib.import_module("trn_agent_boot.trn_boot")
        hook = tb._ntff_profile_via_ctypes("/opt/axon/libaxon_pjrt.so")
        mod = types.ModuleType("antenv.axon_hooks")
        mod.get_axon_ntff_profile_hook = lambda: hook
        _sys.modules["antenv.axon_hooks"] = mod
    import concourse.bass_utils as bu
    bu.upload_artifacts = lambda tmpdir: "(upload disabled)"


_ldw_patched = [False]


def _maybe_enable_ldw_opt():
    """Opt-in experiment: flip walrus --enable-ldw-opt to true."""
    if _ldw_patched[0] or not os.environ.get("KBENCH_LDW_OPT"):
        return
    import concourse.bass_utils as bu
    orig = bu.run_command

    def patched(cmd, *a, **kw):
        cmd = [("--enable-ldw-opt=true" if c == "--enable-ldw-opt=false"
                else c) for c in cmd]
        return orig(cmd, *a, **kw)

    bu.run_command = patched
    _ldw_patched[0] = True


def kernel(**inputs):
    global last_result
    _maybe_enable_ldw_opt()
    args = {k: np.asarray(v, dtype=np.float32) for k, v in inputs.items()}
    in_maps, metas = _prep_inputs(
        args["x"], args["ln1_w"], args["ln1_b"], args["wqkv"], args["bqkv"],
        args["wo"], args["bo"], args["ln2_w"], args["ln2_b"],
        args["w1"], args["b1"], args["w2"], args["b2"])
    nc = _get_nc()
    kwargs = {}
    if os.environ.get("KBENCH_TRACE"):
        _install_ntff_hook_shim()
        kwargs = dict(trace=True,
                      trace_cores=[int(c) for c in
                                   os.environ.get("KBENCH_TRACE_CORES",
                                                  "0").split(",")])
    res = run_bass_kernel_spmd(nc, in_maps, core_ids=list(range(NCORES)),
                               **kwargs)
    last_result = res
    out = np.empty((B, S, D), dtype=np.float32)
    for c in range(NCORES):
        batch, tok = metas[c]
        out[batch, tok] = res.results[c]["out"]
    return out

